# revision 45
# baseline (speedup 1.0000x reference)
"""Bass/Trainium2 kernel for nn_Net_19602230739296 (NNConv + GRU message passing GNN).

Algorithm (mathematically equivalent to the reference, fp32 everywhere):
  theta[e] = (edge_attr[e] @ nn_w + nn_b).reshape(H, H) is never materialized.
  msg[e]   = sum_c ea'[e,c] * (out[src_e] @ W_c)   with ea' = [edge_attr, 1],
             W_c = nn_w[c].reshape(H,H) for c<4, W_4 = nn_b.reshape(H,H).
  agg^T    = sum_c W_c^T @ (G^T @ Q_c)  per 128-edge tile, where G = out[src]
             (gathered rows) and Q_c[e, slot] = ea'[e,c] * [dst_e == slot-node]
             is a host-precomputed weighted one-hot "scatter" matrix.

Sharding: edges are sorted by destination and packed into tiles of <=128
edges covering <=32 whole destination nodes.  Nodes are renumbered to
(core, local_tile*32 + slot) with tiles STRIPED across cores (tile g ->
core g%8) so each core gets an equal share of dense and padding tiles.
Each core's edges land only in its own node range, so no cross-core
reduction is needed; the evolving node features are replicated via
AllGather each iteration.

fp32 matmul on TRN2 is two passes (LOW/HIGH), so the kernel packs pairs of
64-row contractions into single 128-row contractions wherever possible:
  - node state lives as mh = [h (rows 0:64); m (rows 64:128)], so the GRU's
    r and z gates are ONE matmul with stationary [whh_rz; wih_rz], and
    n1/n2/root use zero-padded 128-row stationaries.
  - matmul2's five channel reductions become three: S columns are stored
    even-channels (c0,c2,c4) on rows 0:64 and odd (c1,c3) on rows 64:128,
    with stationaries [W0;W1], [W2;W3], [W4;0].
"""
import os
import sys

import numpy as np


def _ensure_path():
    for p in ("/opt/trn_rl_repo", os.path.expanduser("~/.axon_site/_ro/trn_rl_repo")):
        if os.path.isdir(p) and p not in sys.path:
            sys.path.insert(0, p)
    try:
        import concourse  # noqa: F401
    except ImportError as e:  # pragma: no cover
        raise ImportError(f"concourse (bass) not importable: {e}")


_ensure_path()

N_NODES, N_EDGES, IN_F, H = 10000, 50000, 32, 64
NCORES = 8
SLOTS = 32            # destination-node slots per tile
EPT = 128             # edge slots per tile
NCH = 5               # edge_attr channels (4) + constant channel for nn_b
QCT = NCH * SLOTS     # q columns per tile (evens c0,c2,c4 then odds c1,c3)
SCT = 3 * SLOTS       # s2 columns per tile (3 channel-pairs)
T = 56                # tiles per core (fixed so the compiled NEFF is shape-stable)
T_ACT = 53            # tiles per core that can hold real edges (FFD pack: 420
                      # tiles for this graph -> ceil(420/8)=53); tiles beyond
                      # this are pure padding and the edge phase skips them
NTILES = NCORES * T   # 448
NC_COLS = T * SLOTS   # padded nodes per core (1792)
NPAD = NCORES * NC_COLS
CHUNK = 512
# Gather must be chunked: with single_packet=True the whole m2s stream of one
# dma_gather becomes one packet per SDMA engine, and the HW packet ceiling is
# 64 descriptors.  7 tiles -> 896 idxs -> 57 descs/engine.  Rotate chunks
# across the 4 SWDGE queues so descriptor generation runs on 4 Q7 cpu pairs.
GATHER_BOUNDS = (3, 4, 5, 5, 5, 5, 5, 5, 5, 4, 4, 3)  # tiles/chunk, sum=T_ACT,
# len divisible by the 4 SWDGE queues (tc locks DMASW sems per queue)
N_SWDGE_QUEUES = 4
# writeback splits here: the first half's transposes/DMA overlap the GRU tail
HALF_A = 2 * CHUNK


def _chunks():
    out = []
    c0 = 0
    while c0 < NC_COLS:
        w = min(CHUNK, NC_COLS - c0)
        out.append((c0, w))
        c0 += w
    return out


# ----------------------------------------------------------------------------
# device program
# ----------------------------------------------------------------------------
_NC_CACHE = {}


def _get_nc():
    if "nc" in _NC_CACHE:
        return _NC_CACHE["nc"]
    import concourse.bacc as bacc
    import concourse.mybir as mybir
    import concourse.tile as tile

    dt = mybir.dt
    f32, i16 = dt.float32, dt.int16
    AF = mybir.ActivationFunctionType
    ALU = mybir.AluOpType

    nc = bacc.Bacc(
        "TRN2",
        target_bir_lowering=False,
        debug=False,
        enable_asserts=False,
        num_devices=NCORES,
        num_swdge_queues=N_SWDGE_QUEUES,
    )

    q_in = nc.dram_tensor("q_in", [128, T * QCT], f32, kind="ExternalInput").ap()
    idx_in = nc.dram_tensor("idx_in", [128, T * 8], i16, kind="ExternalInput").ap()
    xt_in = nc.dram_tensor("xt_in", [IN_F, NC_COLS], f32, kind="ExternalInput").ap()
    ws2_in = nc.dram_tensor("ws2_in", [128, 3 * H], f32, kind="ExternalInput").ap()
    lin0_in = nc.dram_tensor("lin0_in", [IN_F, H], f32, kind="ExternalInput").ap()
    wpack_in = nc.dram_tensor("wpack_in", [128, 5 * H], f32, kind="ExternalInput").ap()
    bias_in = nc.dram_tensor("bias_in", [128, 6], f32, kind="ExternalInput").ap()
    ident_in = nc.dram_tensor("ident_in", [128, 128], f32, kind="ExternalInput").ap()
    out_ext = nc.dram_tensor("out_sl", [H, NC_COLS], f32, kind="ExternalOutput").ap()

    chunks = _chunks()

    with tile.TileContext(nc) as tc:
        with tc.tile_pool(name="const", bufs=1) as const, \
             tc.tile_pool(name="work", bufs=1) as work, \
             tc.tile_pool(name="small", bufs=2) as small, \
             tc.tile_pool(name="ps", bufs=4, space="PSUM") as ps, \
             tc.tile_pool(name="aggp", bufs=1, space="PSUM") as aggp, \
             tc.tile_pool(name="dram", bufs=1, space="DRAM") as dram:

            q_sb = const.tile([128, T * QCT], f32, name="q_sb")
            idx_sb = const.tile([128, T * 8], i16, name="idx_sb")
            xt_sb = const.tile([IN_F, NC_COLS], f32, name="xt_sb")
            ws2_sb = const.tile([128, 3 * H], f32, name="ws2_sb")
            lin0_sb = const.tile([IN_F, H], f32, name="lin0_sb")
            wpack_sb = const.tile([128, 5 * H], f32, name="wpack_sb")
            bias_sb = const.tile([128, 6], f32, name="bias_sb")
            ident_sb = const.tile([128, 128], f32, name="ident_sb")

            # iteration-0 dependencies first so the first AllGather can
            # trigger while the big q/idx streams are still loading
            for sb_t, in_t in (
                (xt_sb, xt_in), (lin0_sb, lin0_in), (bias_sb, bias_in),
                (ident_sb, ident_in), (ws2_sb, ws2_in), (wpack_sb, wpack_in),
                (idx_sb, idx_in), (q_sb, q_in),
            ):
                nc.sync.dma_start(sb_t[:], in_t[:])

            l0b = bias_sb[0:H, 0:1]
            convb = bias_sb[0:H, 1:2]
            br = bias_sb[0:H, 2:3]
            bnih = bias_sb[0:H, 3:4]
            bnhh = bias_sb[0:H, 4:5]
            bz = bias_sb[0:H, 5:6]

            mh_a = work.tile([128, NC_COLS], f32, name="mh_a")
            mh_b = work.tile([128, NC_COLS], f32, name="mh_b")
            row_sb = work.tile([128, (NC_COLS // 128) * H], f32, name="row_sb")
            g_sb = work.tile([128, T * H], f32, name="g_sb")
            s2_sb = work.tile([128, T * SCT], f32, name="s2_sb")

            # kill NaN risk from uninitialized SBUF that 0-stationaries touch
            # (s2 fully: the odd-channel rows of pair 2 and all inactive-tile
            # columns are never written by the per-iteration copies)
            nc.vector.memset(mh_a[64:128, :], 0.0)
            nc.vector.memset(mh_b[64:128, :], 0.0)
            nc.vector.memset(s2_sb[:], 0.0)

            agins = [dram.tile([NC_COLS, H], f32, name=f"agin{i}") for i in range(3)]
            agouts = [
                dram.tile([NPAD, H], f32, addr_space="Shared", name=f"agout{i}")
                for i in range(3)
            ]

            # tiny dummy collective issued at boot: absorbs the first-use
            # global barrier + stream warmup into the load/iter0 phase
            # instead of delaying the first real AllGather
            warm_in = dram.tile([128, 2], f32, name="warm_in")
            warm_out = dram.tile(
                [NCORES * 128, 2], f32, addr_space="Shared", name="warm"
            )
            nc.sync.dma_start(warm_in[:], bias_sb[:, 0:2])
            nc.gpsimd.collective_compute(
                "AllGather",
                mybir.AluOpType.bypass,
                replica_groups=[list(range(NCORES))],
                ins=[warm_in[:]],
                outs=[warm_out[:]],
            )
            def writeback(mh, i, c_lo, c_hi, ag):
                # transpose h = mh[0:64] -> row-major rows [c_lo, c_hi); the
                # transposes + DMA of the first half overlap the GRU tail,
                # then one AllGather covers the full slice.  The final
                # iteration (i==3) skips the transpose entirely: out_ext is
                # column-major and the host transposes.
                if i == 3:
                    nc.sync.dma_start(out_ext[:, c_lo:c_hi], mh[0:64, c_lo:c_hi])
                    return
                for nt in range(c_lo // 128, c_hi // 128):
                    tp = ps.tile([128, H], f32, tag="w", name=f"tp{i}_{nt}")
                    nc.tensor.transpose(
                        tp[:], mh[0:64, nt * 128:(nt + 1) * 128], ident_sb[:H, :H]
                    )
                    if nt % 2 == 0:
                        nc.scalar.copy(row_sb[:, nt * H:(nt + 1) * H], tp[:])
                    else:
                        nc.vector.tensor_copy(row_sb[:, nt * H:(nt + 1) * H], tp[:])
                nc.sync.dma_start(
                    agins[i][c_lo:c_hi].rearrange("(t p) o -> p t o", p=128),
                    row_sb[:, (c_lo // 128) * H:(c_hi // 128) * H].rearrange(
                        "p (t o) -> p t o", o=H
                    ),
                )
                if ag:
                    nc.gpsimd.collective_compute(
                        "AllGather",
                        mybir.AluOpType.bypass,
                        replica_groups=[list(range(NCORES))],
                        ins=[agins[i][:]],
                        outs=[agouts[i][:]],
                    )

            # ---- iteration 0: h0 = relu(x @ lin0_w + lin0_b) ----
            for k, (c0, w) in enumerate(chunks):
                p0 = ps.tile([64, CHUNK], f32, tag="w", name=f"p0_{k}")
                nc.tensor.matmul(
                    p0[:, :w], lin0_sb[:], xt_sb[:, c0:c0 + w], start=True, stop=True
                )
                nc.scalar.activation(
                    mh_a[0:64, c0:c0 + w], p0[:, :w], AF.Relu, bias=l0b
                )
            writeback(mh_a, 0, 0, NC_COLS, ag=True)

            def edge_phase(it, mh):
                src_dram = agouts[it - 1]
                t0 = 0
                for gc, gsz in enumerate(GATHER_BOUNDS):
                    nc.gpsimd.dma_gather(
                        g_sb[:, t0 * H:(t0 + gsz) * H].rearrange(
                            "p (t o) -> p t o", o=H
                        ),
                        src_dram[:],
                        idx_sb[:, t0 * 8:(t0 + gsz) * 8],
                        gsz * EPT,
                        gsz * EPT,
                        H,
                        queue_num=gc % N_SWDGE_QUEUES,
                    )
                    t0 += gsz
                # matmul1: S_t = G_t^T @ Q_t, 3 tiles per PSUM bank.
                # S columns per tile: evens (c0,c2,c4 | 96) then odds (c1,c3 | 64);
                # the copy to s2_sb stacks odds on partitions 64:128.
                t = 0
                while t < T_ACT:
                    ntl = min(3, T_ACT - t)
                    s_ps = ps.tile([64, CHUNK], f32, tag="w", name=f"sps{it}_{t}")
                    for j in range(ntl):
                        nc.tensor.matmul(
                            s_ps[:, j * QCT:(j + 1) * QCT],
                            g_sb[:, (t + j) * H:(t + j + 1) * H],
                            q_sb[:, (t + j) * QCT:(t + j + 1) * QCT],
                            start=True,
                            stop=True,
                        )
                    src = s_ps[:, :ntl * QCT].rearrange("p (j x) -> p j x", x=QCT)
                    dste = s2_sb[0:64, t * SCT:(t + ntl) * SCT].rearrange(
                        "p (j x) -> p j x", x=SCT
                    )
                    dsto = s2_sb[64:128, t * SCT:(t + ntl) * SCT].rearrange(
                        "p (j x) -> p j x", x=SCT
                    )
                    if (t // 3) % 2 == 0:
                        nc.vector.tensor_copy(dste[:], src[:, :, 0:96])
                        nc.scalar.copy(dsto[:, :, 0:64], src[:, :, 96:160])
                    else:
                        nc.scalar.copy(dste[:], src[:, :, 0:96])
                        nc.vector.tensor_copy(dsto[:, :, 0:64], src[:, :, 96:160])
                    t += ntl
                # matmul2: agg^T += sum over 3 channel-pairs (16 tiles/matmul)
                agg = aggp.tile([64, 4 * CHUNK], f32, tag="agg", name=f"agg{it}")
                s2_re = s2_sb[:].rearrange("p (t q s) -> p t q s", q=3, s=SLOTS)
                for p in range(3):
                    g0 = 0
                    while g0 < T_ACT:
                        gn = min(16, T_ACT - g0)
                        nc.tensor.matmul(
                            agg[:, g0 * SLOTS:(g0 + gn) * SLOTS],
                            ws2_sb[:, p * H:(p + 1) * H],
                            s2_re[:, g0:g0 + gn, p, :],
                            start=(p == 0),
                            stop=False,
                        )
                        g0 += gn
                # += root_w^T h  (stationary rows 64:128 are zero -> m ignored).
                # Columns past the active-tile region got no matmul2 writes, so
                # the root matmul starts the PSUM accumulation there.
                act_end = T_ACT * SLOTS
                for c0, w in chunks:
                    if c0 + w <= act_end:
                        spans = ((c0, w, False),)
                    elif c0 >= act_end:
                        spans = ((c0, w, True),)
                    else:
                        spans = (
                            (c0, act_end - c0, False),
                            (act_end, c0 + w - act_end, True),
                        )
                    for s0, sw, st in spans:
                        nc.tensor.matmul(
                            agg[:, s0:s0 + sw], wpack_sb[:, 256:320],
                            mh[:, s0:s0 + sw], start=st, stop=True,
                        )
                return agg

            def dense_gru(agg, mh, mh_next, it, ks):
                for k in ks:
                    c0, w = chunks[k]
                    # m = relu(agg + conv_b) into the m-half of mh
                    nc.scalar.activation(
                        mh[64:128, c0:c0 + w], agg[:, c0:c0 + w], AF.Relu, bias=convb
                    )
                    # r and z gates in one 128-row matmul:
                    #   rz = [whh_rz; wih_rz]^T @ [h; m]
                    rz = ps.tile([128, CHUNK], f32, tag="w", name=f"rz{it}_{k}")
                    nc.tensor.matmul(
                        rz[:, :w], wpack_sb[:, 0:128], mh[:, c0:c0 + w],
                        start=True, stop=True,
                    )
                    r_sb = small.tile([64, CHUNK], f32, tag="rsb", name=f"rs{it}{k}")
                    nc.scalar.activation(
                        r_sb[:, :w], rz[0:64, :w], AF.Sigmoid, bias=br
                    )
                    z_sb = small.tile([64, CHUNK], f32, tag="zsb", name=f"zs{it}{k}")
                    nc.scalar.activation(
                        z_sb[:, :w], rz[64:128, :w], AF.Sigmoid, bias=bz
                    )
                    n1 = ps.tile([64, CHUNK], f32, tag="w", name=f"n1{it}_{k}")
                    nc.tensor.matmul(
                        n1[:, :w], wpack_sb[:, 128:192], mh[:, c0:c0 + w],
                        start=True, stop=True,
                    )
                    n2 = ps.tile([64, CHUNK], f32, tag="w", name=f"n2{it}_{k}")
                    nc.tensor.matmul(
                        n2[:, :w], wpack_sb[:, 192:256], mh[:, c0:c0 + w],
                        start=True, stop=True,
                    )
                    # tmp = (n2 + b_hh_n) * r
                    tmp = small.tile([64, CHUNK], f32, tag="tmp", name=f"tmp{it}{k}")
                    nc.vector.scalar_tensor_tensor(
                        tmp[:, :w], n2[:, :w], bnhh, r_sb[:, :w], ALU.add, ALU.mult
                    )
                    pre = small.tile([64, CHUNK], f32, tag="pre", name=f"pre{it}{k}")
                    nc.vector.tensor_add(pre[:, :w], n1[:, :w], tmp[:, :w])
                    nsb = small.tile([64, CHUNK], f32, tag="nsb", name=f"nsb{it}{k}")
                    nc.scalar.activation(nsb[:, :w], pre[:, :w], AF.Tanh, bias=bnih)
                    # h' = n + z * (h - n)
                    dd = small.tile([64, CHUNK], f32, tag="dd", name=f"dd{it}{k}")
                    nc.vector.tensor_sub(dd[:, :w], mh[0:64, c0:c0 + w], nsb[:, :w])
                    t4 = small.tile([64, CHUNK], f32, tag="t4", name=f"t4{it}{k}")
                    nc.vector.tensor_mul(t4[:, :w], z_sb[:, :w], dd[:, :w])
                    nc.vector.tensor_add(
                        mh_next[0:64, c0:c0 + w], nsb[:, :w], t4[:, :w]
                    )

            mh, mh_next = mh_a, mh_b
            for it in (1, 2):
                agg = edge_phase(it, mh)
                dense_gru(agg, mh, mh_next, it, (0, 1))
                writeback(mh_next, it, 0, HALF_A, ag=False)
                dense_gru(agg, mh, mh_next, it, (2, 3))
                writeback(mh_next, it, HALF_A, NC_COLS, ag=True)
                mh, mh_next = mh_next, mh
            # final iteration: per-chunk column-major DMA, no transposes
            agg = edge_phase(3, mh)
            for k, (c0, w) in enumerate(chunks):
                dense_gru(agg, mh, mh_next, 3, (k,))
                writeback(mh_next, 3, c0, c0 + w, ag=False)

    nc.compile()
    _NC_CACHE["nc"] = nc
    return nc


# ----------------------------------------------------------------------------
# host-side graph preprocessing (pure data layout, no model FLOPs)
# ----------------------------------------------------------------------------
def _pack(edge_index, edge_attr):
    src = np.asarray(edge_index[0]).astype(np.int64)
    dst = np.asarray(edge_index[1]).astype(np.int64)
    ea = np.asarray(edge_attr, np.float32)
    order = np.argsort(dst, kind="stable")
    ssrc, sea = src[order], ea[order]
    deg = np.bincount(dst, minlength=N_NODES)
    starts = np.zeros(N_NODES + 1, np.int64)
    starts[1:] = np.cumsum(deg)
    uniq = np.flatnonzero(deg)
    zs = np.flatnonzero(deg == 0)
    node_seq = np.concatenate([uniq, zs])

    # first-fit decreasing bin packing: bins capped at SLOTS nodes / EPT
    # edges; zero-degree nodes land in the node-slack of edge-full bins
    order = np.argsort(-deg[node_seq], kind="stable")
    tiles_nodes = []
    tile_e, tile_n = [], []
    for nd in node_seq[order]:
        d = int(deg[nd])
        assert d <= EPT, f"node degree {d} exceeds edge tile capacity"
        for b in range(len(tiles_nodes)):
            if tile_n[b] < SLOTS and tile_e[b] + d <= EPT:
                tiles_nodes[b].append(int(nd))
                tile_e[b] += d
                tile_n[b] += 1
                break
        else:
            tiles_nodes.append([int(nd)])
            tile_e.append(d)
            tile_n.append(1)
    assert len(tiles_nodes) <= T_ACT * NCORES, (
        f"need {len(tiles_nodes)} tiles > {T_ACT * NCORES}"
    )

    # Tiles are striped across cores (tile g -> core g%8, local slot g//8) so
    # every core gets an equal share of dense and padding tiles: the trailing
    # all-padding tiles otherwise all land on the last core, whose gather
    # then serializes and stalls every AllGather by ~85us.
    perm = np.empty(N_NODES, np.int64)
    for t, nodes in enumerate(tiles_nodes):
        col = (t % NCORES) * NC_COLS + (t // NCORES) * SLOTS
        for j, nd in enumerate(nodes):
            perm[nd] = col + j

    q = np.zeros((NTILES, EPT, NCH, SLOTS), np.float32)
    srcslot = np.zeros((NTILES, EPT), np.int16)
    e_arange = np.arange(EPT, dtype=np.int64)
    for t in range(NTILES):
        e = 0
        for j, nd in enumerate(tiles_nodes[t] if t < len(tiles_nodes) else ()):
            s0, s1 = int(starts[nd]), int(starts[nd + 1])
            ne = s1 - s0
            if ne:
                q[t, e:e + ne, 0:4, j] = sea[s0:s1]
                q[t, e:e + ne, 4, j] = 1.0
                srcslot[t, e:e + ne] = perm[ssrc[s0:s1]].astype(np.int16)
                e += ne
        # padding rows: q is zero so the gathered values are irrelevant, but
        # give each row a distinct consecutive address -- identical indices
        # (all-zero) serialize the gather on a single HBM row
        if e < EPT:
            srcslot[t, e:] = ((t * EPT + e_arange[e:]) % NPAD).astype(np.int16)

    # channel order evens-then-odds so matmul2 can stack channel pairs
    q = q[:, :, [0, 2, 4, 1, 3], :]

    qs, idxs = [], []
    i_arange = np.arange(T * EPT)
    for k in range(NCORES):
        qt = q[k::NCORES]
        qs.append(
            np.ascontiguousarray(qt.transpose(1, 0, 2, 3)).reshape(128, T * QCT)
        )
        flat = srcslot[k::NCORES].reshape(-1)
        ia = np.zeros((128, T * 8), np.int16)
        # the index list is read per 16-partition group by each of the 8
        # GPSIMD cores on HW -> replicate it into every group
        for g in range(8):
            ia[g * 16 + i_arange % 16, i_arange // 16] = flat
        idxs.append(ia)
    return qs, idxs, perm


def _prep_inputs(inputs):
    x = np.asarray(inputs["x"], np.float32)
    qs, idxs, perm = _pack(inputs["edge_index"], inputs["edge_attr"])

    x_pad = np.zeros((NPAD, IN_F), np.float32)
    x_pad[perm] = x
    xts = [
        np.ascontiguousarray(x_pad[k * NC_COLS:(k + 1) * NC_COLS].T)
        for k in range(NCORES)
    ]

    nw = np.asarray(inputs["nn_w"], np.float32)
    wc = [nw[c].reshape(H, H) for c in range(4)]
    wc.append(np.asarray(inputs["nn_b"], np.float32).reshape(H, H))
    # matmul2 stationaries: pair p contracts evens on rows 0:64, odds on 64:128
    ws2 = np.zeros((128, 3 * H), np.float32)
    for p, (ce, co) in enumerate(((0, 1), (2, 3), (4, None))):
        ws2[0:H, p * H:(p + 1) * H] = wc[ce]
        if co is not None:
            ws2[H:128, p * H:(p + 1) * H] = wc[co]

    lin0_w = np.ascontiguousarray(np.asarray(inputs["lin0_w"], np.float32))
    root_w = np.asarray(inputs["root_w"], np.float32)
    wih_t = np.asarray(inputs["gru_w_ih"], np.float32).T  # [H, 3H]
    whh_t = np.asarray(inputs["gru_w_hh"], np.float32).T
    b_ih = np.asarray(inputs["gru_b_ih"], np.float32)
    b_hh = np.asarray(inputs["gru_b_hh"], np.float32)

    # packed 128-row stationaries vs mh = [h; m]:
    #   cols 0:128   rz gates  [whh_r|whh_z ; wih_r|wih_z]
    #   cols 128:192 n1 = wih_n @ m   [0 ; wih_n]
    #   cols 192:256 n2 = whh_n @ h   [whh_n ; 0]
    #   cols 256:320 root          [root_w ; 0]
    wpack = np.zeros((128, 5 * H), np.float32)
    wpack[0:H, 0:128] = whh_t[:, 0:128]
    wpack[H:128, 0:128] = wih_t[:, 0:128]
    wpack[H:128, 128:192] = wih_t[:, 128:192]
    wpack[0:H, 192:256] = whh_t[:, 128:192]
    wpack[0:H, 256:320] = root_w

    bias_pack = np.zeros((128, 6), np.float32)
    bias_pack[0:H, 0] = np.asarray(inputs["lin0_b"], np.float32)
    bias_pack[0:H, 1] = np.asarray(inputs["conv_b"], np.float32)
    bias_pack[0:H, 2] = (b_ih + b_hh)[0:64]
    bias_pack[0:H, 3] = b_ih[128:192]
    bias_pack[0:H, 4] = b_hh[128:192]
    bias_pack[0:H, 5] = (b_ih + b_hh)[64:128]
    ident = np.eye(128, dtype=np.float32)

    in_maps = []
    for k in range(NCORES):
        in_maps.append(
            {
                "q_in": qs[k],
                "idx_in": idxs[k],
                "xt_in": xts[k],
                "ws2_in": ws2,
                "lin0_in": lin0_w,
                "wpack_in": wpack,
                "bias_in": bias_pack,
                "ident_in": ident,
            }
        )
    return in_maps, perm


def _assemble(results, perm):
    # out_sl is column-major [H, NC_COLS]; transpose on the host
    full = np.concatenate(
        [results[k]["out_sl"].T for k in range(NCORES)], axis=0
    )
    return np.ascontiguousarray(full[perm]).astype(np.float32)


def kernel(**inputs) -> np.ndarray:
    in_maps, perm = _prep_inputs(inputs)
    nc = _get_nc()
    if os.environ.get("BASS_KERNEL_SIM"):
        results = _run_sim(nc, in_maps)
    else:
        from concourse import bass_utils

        res = bass_utils.run_bass_kernel_spmd(
            nc, in_maps, core_ids=list(range(NCORES))
        )
        results = res.results
    return _assemble(results, perm)


def _run_sim(nc, in_maps):
    from concourse.bass_interp import MultiCoreSim

    sim = MultiCoreSim(nc, num_cores=NCORES, trace=False)
    for k, core in sim.cores.items():
        for name, arr in in_maps[k].items():
            core.tensor(name)[:] = arr
    sim.simulate(check_with_hw=False)
    out = []
    for k in range(NCORES):
        out.append({"out_sl": np.array(sim.cores[k].tensor("out_sl"))})
    return out


if __name__ == "__main__":
    rng = np.random.default_rng(0)
    demo = {
        "x": rng.standard_normal((N_NODES, IN_F), dtype=np.float32),
        "edge_index": rng.integers(0, N_NODES, (2, N_EDGES)).astype(np.int32),
        "edge_attr": rng.random((N_EDGES, 4), dtype=np.float32),
        "lin0_w": rng.standard_normal((IN_F, H), dtype=np.float32) * 0.1,
        "lin0_b": np.zeros(H, np.float32),
        "nn_w": rng.standard_normal((4, H * H), dtype=np.float32) * 0.05,
        "nn_b": np.zeros(H * H, np.float32),
        "root_w": rng.standard_normal((H, H), dtype=np.float32) * 0.1,
        "conv_b": np.zeros(H, np.float32),
        "gru_w_ih": rng.standard_normal((3 * H, H), dtype=np.float32) * 0.1,
        "gru_w_hh": rng.standard_normal((3 * H, H), dtype=np.float32) * 0.1,
        "gru_b_ih": np.zeros(3 * H, np.float32),
        "gru_b_hh": np.zeros(3 * H, np.float32),
    }
    out = kernel(**demo)
    print("kernel output", out.shape, out.dtype, float(np.abs(out).mean()))


# revision 46
# speedup vs baseline: 1.0002x; 1.0002x over previous
"""Bass/Trainium2 kernel for nn_Net_19602230739296 (NNConv + GRU message passing GNN).

Algorithm (mathematically equivalent to the reference, fp32 everywhere):
  theta[e] = (edge_attr[e] @ nn_w + nn_b).reshape(H, H) is never materialized.
  msg[e]   = sum_c ea'[e,c] * (out[src_e] @ W_c)   with ea' = [edge_attr, 1],
             W_c = nn_w[c].reshape(H,H) for c<4, W_4 = nn_b.reshape(H,H).
  agg^T    = sum_c W_c^T @ (G^T @ Q_c)  per 128-edge tile, where G = out[src]
             (gathered rows) and Q_c[e, slot] = ea'[e,c] * [dst_e == slot-node]
             is a host-precomputed weighted one-hot "scatter" matrix.

Sharding: edges are sorted by destination and packed into tiles of <=128
edges covering <=32 whole destination nodes.  Nodes are renumbered to
(core, local_tile*32 + slot) with tiles STRIPED across cores (tile g ->
core g%8) so each core gets an equal share of dense and padding tiles.
Each core's edges land only in its own node range, so no cross-core
reduction is needed; the evolving node features are replicated via
AllGather each iteration.

fp32 matmul on TRN2 is two passes (LOW/HIGH), so the kernel packs pairs of
64-row contractions into single 128-row contractions wherever possible:
  - node state lives as mh = [h (rows 0:64); m (rows 64:128)], so the GRU's
    r and z gates are ONE matmul with stationary [whh_rz; wih_rz], and
    n1/n2/root use zero-padded 128-row stationaries.
  - matmul2's five channel reductions become three: S columns are stored
    even-channels (c0,c2,c4) on rows 0:64 and odd (c1,c3) on rows 64:128,
    with stationaries [W0;W1], [W2;W3], [W4;0].
"""
import os
import sys

import numpy as np


def _ensure_path():
    for p in ("/opt/trn_rl_repo", os.path.expanduser("~/.axon_site/_ro/trn_rl_repo")):
        if os.path.isdir(p) and p not in sys.path:
            sys.path.insert(0, p)
    try:
        import concourse  # noqa: F401
    except ImportError as e:  # pragma: no cover
        raise ImportError(f"concourse (bass) not importable: {e}")


_ensure_path()

N_NODES, N_EDGES, IN_F, H = 10000, 50000, 32, 64
NCORES = 8
SLOTS = 32            # destination-node slots per tile
EPT = 128             # edge slots per tile
NCH = 5               # edge_attr channels (4) + constant channel for nn_b
QCT = NCH * SLOTS     # q columns per tile (evens c0,c2,c4 then odds c1,c3)
SCT = 3 * SLOTS       # s2 columns per tile (3 channel-pairs)
T = 56                # tiles per core (fixed so the compiled NEFF is shape-stable)
T_ACT = 53            # tiles per core that can hold real edges (FFD pack: 420
                      # tiles for this graph -> ceil(420/8)=53); tiles beyond
                      # this are pure padding and the edge phase skips them
NTILES = NCORES * T   # 448
NC_COLS = T * SLOTS   # padded nodes per core (1792)
NPAD = NCORES * NC_COLS
CHUNK = 512
# Gather must be chunked: with single_packet=True the whole m2s stream of one
# dma_gather becomes one packet per SDMA engine, and the HW packet ceiling is
# 64 descriptors.  7 tiles -> 896 idxs -> 57 descs/engine.  Rotate chunks
# across the 4 SWDGE queues so descriptor generation runs on 4 Q7 cpu pairs.
GATHER_BOUNDS = (3, 4, 5, 5, 5, 5, 5, 5, 5, 4, 4, 3)  # tiles/chunk, sum=T_ACT,
# len divisible by the 4 SWDGE queues (tc locks DMASW sems per queue)
N_SWDGE_QUEUES = 4
# writeback splits here: the first half's transposes/DMA overlap the GRU tail
HALF_A = 2 * CHUNK


def _chunks():
    out = []
    c0 = 0
    while c0 < NC_COLS:
        w = min(CHUNK, NC_COLS - c0)
        out.append((c0, w))
        c0 += w
    return out


# ----------------------------------------------------------------------------
# device program
# ----------------------------------------------------------------------------
_NC_CACHE = {}


def _get_nc():
    if "nc" in _NC_CACHE:
        return _NC_CACHE["nc"]
    import concourse.bacc as bacc
    import concourse.mybir as mybir
    import concourse.tile as tile

    dt = mybir.dt
    f32, i16 = dt.float32, dt.int16
    AF = mybir.ActivationFunctionType
    ALU = mybir.AluOpType

    nc = bacc.Bacc(
        "TRN2",
        target_bir_lowering=False,
        debug=False,
        enable_asserts=False,
        num_devices=NCORES,
        num_swdge_queues=N_SWDGE_QUEUES,
    )

    q_in = nc.dram_tensor("q_in", [128, T * QCT], f32, kind="ExternalInput").ap()
    idx_in = nc.dram_tensor("idx_in", [128, T * 8], i16, kind="ExternalInput").ap()
    xt_in = nc.dram_tensor("xt_in", [IN_F, NC_COLS], f32, kind="ExternalInput").ap()
    ws2_in = nc.dram_tensor("ws2_in", [128, 3 * H], f32, kind="ExternalInput").ap()
    lin0_in = nc.dram_tensor("lin0_in", [IN_F, H], f32, kind="ExternalInput").ap()
    wpack_in = nc.dram_tensor("wpack_in", [128, 5 * H], f32, kind="ExternalInput").ap()
    bias_in = nc.dram_tensor("bias_in", [128, 6], f32, kind="ExternalInput").ap()
    ident_in = nc.dram_tensor("ident_in", [128, 128], f32, kind="ExternalInput").ap()
    out_ext = nc.dram_tensor("out_sl", [H, NC_COLS], f32, kind="ExternalOutput").ap()

    chunks = _chunks()

    with tile.TileContext(nc) as tc:
        with tc.tile_pool(name="const", bufs=1) as const, \
             tc.tile_pool(name="work", bufs=1) as work, \
             tc.tile_pool(name="small", bufs=2) as small, \
             tc.tile_pool(name="ps", bufs=4, space="PSUM") as ps, \
             tc.tile_pool(name="aggp", bufs=1, space="PSUM") as aggp, \
             tc.tile_pool(name="dram", bufs=1, space="DRAM") as dram:

            q_sb = const.tile([128, T * QCT], f32, name="q_sb")
            idx_sb = const.tile([128, T * 8], i16, name="idx_sb")
            xt_sb = const.tile([IN_F, NC_COLS], f32, name="xt_sb")
            ws2_sb = const.tile([128, 3 * H], f32, name="ws2_sb")
            lin0_sb = const.tile([IN_F, H], f32, name="lin0_sb")
            wpack_sb = const.tile([128, 5 * H], f32, name="wpack_sb")
            bias_sb = const.tile([128, 6], f32, name="bias_sb")
            ident_sb = const.tile([128, 128], f32, name="ident_sb")

            # iteration-0 dependencies first so the first AllGather can
            # trigger while the big q/idx streams are still loading
            for sb_t, in_t in (
                (xt_sb, xt_in), (lin0_sb, lin0_in), (bias_sb, bias_in),
                (ident_sb, ident_in), (ws2_sb, ws2_in), (wpack_sb, wpack_in),
                (idx_sb, idx_in), (q_sb, q_in),
            ):
                nc.sync.dma_start(sb_t[:], in_t[:])

            l0b = bias_sb[0:H, 0:1]
            convb = bias_sb[0:H, 1:2]
            br = bias_sb[0:H, 2:3]
            bnih = bias_sb[0:H, 3:4]
            bnhh = bias_sb[0:H, 4:5]
            bz = bias_sb[0:H, 5:6]

            mh_a = work.tile([128, NC_COLS], f32, name="mh_a")
            mh_b = work.tile([128, NC_COLS], f32, name="mh_b")
            row_sb = work.tile([128, (NC_COLS // 128) * H], f32, name="row_sb")
            g_sb = work.tile([128, T * H], f32, name="g_sb")
            s2_sb = work.tile([128, T * SCT], f32, name="s2_sb")

            # kill NaN risk from uninitialized SBUF that 0-stationaries touch
            # (s2 fully: the odd-channel rows of pair 2 and all inactive-tile
            # columns are never written by the per-iteration copies)
            nc.vector.memset(mh_a[64:128, :], 0.0)
            nc.vector.memset(mh_b[64:128, :], 0.0)
            nc.vector.memset(s2_sb[:], 0.0)

            agins = [dram.tile([NC_COLS, H], f32, name=f"agin{i}") for i in range(3)]
            agouts = [
                dram.tile([NPAD, H], f32, addr_space="Shared", name=f"agout{i}")
                for i in range(3)
            ]

            # tiny dummy collective issued at boot: absorbs the first-use
            # global barrier + stream warmup into the load/iter0 phase
            # instead of delaying the first real AllGather
            warm_in = dram.tile([128, 2], f32, name="warm_in")
            warm_out = dram.tile(
                [NCORES * 128, 2], f32, addr_space="Shared", name="warm"
            )
            nc.sync.dma_start(warm_in[:], bias_sb[:, 0:2])
            nc.gpsimd.collective_compute(
                "AllGather",
                mybir.AluOpType.bypass,
                replica_groups=[list(range(NCORES))],
                ins=[warm_in[:]],
                outs=[warm_out[:]],
            )
            def writeback(mh, i, c_lo, c_hi, ag):
                # transpose h = mh[0:64] -> row-major rows [c_lo, c_hi); the
                # transposes + DMA of the first half overlap the GRU tail,
                # then one AllGather covers the full slice.  The final
                # iteration (i==3) skips the transpose entirely: out_ext is
                # column-major and the host transposes.
                if i == 3:
                    nc.sync.dma_start(out_ext[:, c_lo:c_hi], mh[0:64, c_lo:c_hi])
                    return
                for nt in range(c_lo // 128, c_hi // 128):
                    tp = ps.tile([128, H], f32, tag="w", name=f"tp{i}_{nt}")
                    nc.tensor.transpose(
                        tp[:], mh[0:64, nt * 128:(nt + 1) * 128], ident_sb[:H, :H]
                    )
                    if nt % 2 == 0:
                        nc.scalar.copy(row_sb[:, nt * H:(nt + 1) * H], tp[:])
                    else:
                        nc.vector.tensor_copy(row_sb[:, nt * H:(nt + 1) * H], tp[:])
                nc.sync.dma_start(
                    agins[i][c_lo:c_hi].rearrange("(t p) o -> p t o", p=128),
                    row_sb[:, (c_lo // 128) * H:(c_hi // 128) * H].rearrange(
                        "p (t o) -> p t o", o=H
                    ),
                )
                if ag:
                    nc.gpsimd.collective_compute(
                        "AllGather",
                        mybir.AluOpType.bypass,
                        replica_groups=[list(range(NCORES))],
                        ins=[agins[i][:]],
                        outs=[agouts[i][:]],
                    )

            # ---- iteration 0: h0 = relu(x @ lin0_w + lin0_b) ----
            for k, (c0, w) in enumerate(chunks):
                p0 = ps.tile([64, CHUNK], f32, tag="w", name=f"p0_{k}")
                nc.tensor.matmul(
                    p0[:, :w], lin0_sb[:], xt_sb[:, c0:c0 + w], start=True, stop=True
                )
                nc.scalar.activation(
                    mh_a[0:64, c0:c0 + w], p0[:, :w], AF.Relu, bias=l0b
                )
            writeback(mh_a, 0, 0, NC_COLS, ag=True)

            def edge_phase(it, mh):
                src_dram = agouts[it - 1]
                t0 = 0
                for gc, gsz in enumerate(GATHER_BOUNDS):
                    nc.gpsimd.dma_gather(
                        g_sb[:, t0 * H:(t0 + gsz) * H].rearrange(
                            "p (t o) -> p t o", o=H
                        ),
                        src_dram[:],
                        idx_sb[:, t0 * 8:(t0 + gsz) * 8],
                        gsz * EPT,
                        gsz * EPT,
                        H,
                        queue_num=gc % N_SWDGE_QUEUES,
                    )
                    t0 += gsz
                # matmul1: S_t = G_t^T @ Q_t, 3 tiles per PSUM bank.
                # S columns per tile: evens (c0,c2,c4 | 96) then odds (c1,c3 | 64);
                # the copy to s2_sb stacks odds on partitions 64:128.
                t = 0
                while t < T_ACT:
                    ntl = min(3, T_ACT - t)
                    s_ps = ps.tile([64, CHUNK], f32, tag="w", name=f"sps{it}_{t}")
                    for j in range(ntl):
                        nc.tensor.matmul(
                            s_ps[:, j * QCT:(j + 1) * QCT],
                            g_sb[:, (t + j) * H:(t + j + 1) * H],
                            q_sb[:, (t + j) * QCT:(t + j + 1) * QCT],
                            start=True,
                            stop=True,
                        )
                    src = s_ps[:, :ntl * QCT].rearrange("p (j x) -> p j x", x=QCT)
                    dste = s2_sb[0:64, t * SCT:(t + ntl) * SCT].rearrange(
                        "p (j x) -> p j x", x=SCT
                    )
                    dsto = s2_sb[64:128, t * SCT:(t + ntl) * SCT].rearrange(
                        "p (j x) -> p j x", x=SCT
                    )
                    if (t // 3) % 2 == 0:
                        nc.vector.tensor_copy(dste[:], src[:, :, 0:96])
                        nc.scalar.copy(dsto[:, :, 0:64], src[:, :, 96:160])
                    else:
                        nc.scalar.copy(dste[:], src[:, :, 0:96])
                        nc.vector.tensor_copy(dsto[:, :, 0:64], src[:, :, 96:160])
                    t += ntl
                # matmul2: agg^T += sum over 3 channel-pairs (16 tiles/matmul)
                agg = aggp.tile([64, 4 * CHUNK], f32, tag="agg", name=f"agg{it}")
                s2_re = s2_sb[:].rearrange("p (t q s) -> p t q s", q=3, s=SLOTS)
                for p in range(3):
                    g0 = 0
                    while g0 < T_ACT:
                        gn = min(16, T_ACT - g0)
                        nc.tensor.matmul(
                            agg[:, g0 * SLOTS:(g0 + gn) * SLOTS],
                            ws2_sb[:, p * H:(p + 1) * H],
                            s2_re[:, g0:g0 + gn, p, :],
                            start=(p == 0),
                            stop=False,
                        )
                        g0 += gn
                # += root_w^T h  (stationary rows 64:128 are zero -> m ignored).
                # Columns past the active-tile region got no matmul2 writes, so
                # the root matmul starts the PSUM accumulation there.
                act_end = T_ACT * SLOTS
                for c0, w in chunks:
                    if c0 + w <= act_end:
                        spans = ((c0, w, False),)
                    elif c0 >= act_end:
                        spans = ((c0, w, True),)
                    else:
                        spans = (
                            (c0, act_end - c0, False),
                            (act_end, c0 + w - act_end, True),
                        )
                    for s0, sw, st in spans:
                        nc.tensor.matmul(
                            agg[:, s0:s0 + sw], wpack_sb[:, 256:320],
                            mh[:, s0:s0 + sw], start=st, stop=True,
                        )
                return agg

            def dense_gru(agg, mh, mh_next, it, ks):
                for k in ks:
                    c0, w = chunks[k]
                    # m = relu(agg + conv_b) into the m-half of mh
                    nc.scalar.activation(
                        mh[64:128, c0:c0 + w], agg[:, c0:c0 + w], AF.Relu, bias=convb
                    )
                    # r and z gates in one 128-row matmul:
                    #   rz = [whh_rz; wih_rz]^T @ [h; m]
                    rz = ps.tile([128, CHUNK], f32, tag="w", name=f"rz{it}_{k}")
                    nc.tensor.matmul(
                        rz[:, :w], wpack_sb[:, 0:128], mh[:, c0:c0 + w],
                        start=True, stop=True,
                    )
                    r_sb = small.tile([64, CHUNK], f32, tag="rsb", name=f"rs{it}{k}")
                    nc.scalar.activation(
                        r_sb[:, :w], rz[0:64, :w], AF.Sigmoid, bias=br
                    )
                    z_sb = small.tile([64, CHUNK], f32, tag="zsb", name=f"zs{it}{k}")
                    nc.scalar.activation(
                        z_sb[:, :w], rz[64:128, :w], AF.Sigmoid, bias=bz
                    )
                    # n1 and n2 in one 128-col-stationary matmul:
                    #   n12 = [ [0;wih_n] | [whh_n;0] ]^T @ [h; m]
                    n12 = ps.tile([128, CHUNK], f32, tag="w", name=f"n12{it}_{k}")
                    nc.tensor.matmul(
                        n12[:, :w], wpack_sb[:, 128:256], mh[:, c0:c0 + w],
                        start=True, stop=True,
                    )
                    # tmp = (n2 + b_hh_n) * r
                    tmp = small.tile([64, CHUNK], f32, tag="tmp", name=f"tmp{it}{k}")
                    nc.vector.scalar_tensor_tensor(
                        tmp[:, :w], n12[64:128, :w], bnhh, r_sb[:, :w],
                        ALU.add, ALU.mult,
                    )
                    pre = small.tile([64, CHUNK], f32, tag="pre", name=f"pre{it}{k}")
                    nc.vector.tensor_add(pre[:, :w], n12[0:64, :w], tmp[:, :w])
                    nsb = small.tile([64, CHUNK], f32, tag="nsb", name=f"nsb{it}{k}")
                    nc.scalar.activation(nsb[:, :w], pre[:, :w], AF.Tanh, bias=bnih)
                    # h' = n + z * (h - n)
                    dd = small.tile([64, CHUNK], f32, tag="dd", name=f"dd{it}{k}")
                    nc.vector.tensor_sub(dd[:, :w], mh[0:64, c0:c0 + w], nsb[:, :w])
                    t4 = small.tile([64, CHUNK], f32, tag="t4", name=f"t4{it}{k}")
                    nc.vector.tensor_mul(t4[:, :w], z_sb[:, :w], dd[:, :w])
                    nc.vector.tensor_add(
                        mh_next[0:64, c0:c0 + w], nsb[:, :w], t4[:, :w]
                    )

            mh, mh_next = mh_a, mh_b
            for it in (1, 2):
                agg = edge_phase(it, mh)
                dense_gru(agg, mh, mh_next, it, (0, 1))
                writeback(mh_next, it, 0, HALF_A, ag=False)
                dense_gru(agg, mh, mh_next, it, (2, 3))
                writeback(mh_next, it, HALF_A, NC_COLS, ag=True)
                mh, mh_next = mh_next, mh
            # final iteration: per-chunk column-major DMA, no transposes
            agg = edge_phase(3, mh)
            for k, (c0, w) in enumerate(chunks):
                dense_gru(agg, mh, mh_next, 3, (k,))
                writeback(mh_next, 3, c0, c0 + w, ag=False)

    nc.compile()
    _NC_CACHE["nc"] = nc
    return nc


# ----------------------------------------------------------------------------
# host-side graph preprocessing (pure data layout, no model FLOPs)
# ----------------------------------------------------------------------------
def _pack(edge_index, edge_attr):
    src = np.asarray(edge_index[0]).astype(np.int64)
    dst = np.asarray(edge_index[1]).astype(np.int64)
    ea = np.asarray(edge_attr, np.float32)
    order = np.argsort(dst, kind="stable")
    ssrc, sea = src[order], ea[order]
    deg = np.bincount(dst, minlength=N_NODES)
    starts = np.zeros(N_NODES + 1, np.int64)
    starts[1:] = np.cumsum(deg)
    uniq = np.flatnonzero(deg)
    zs = np.flatnonzero(deg == 0)
    node_seq = np.concatenate([uniq, zs])

    # first-fit decreasing bin packing: bins capped at SLOTS nodes / EPT
    # edges; zero-degree nodes land in the node-slack of edge-full bins
    order = np.argsort(-deg[node_seq], kind="stable")
    tiles_nodes = []
    tile_e, tile_n = [], []
    for nd in node_seq[order]:
        d = int(deg[nd])
        assert d <= EPT, f"node degree {d} exceeds edge tile capacity"
        for b in range(len(tiles_nodes)):
            if tile_n[b] < SLOTS and tile_e[b] + d <= EPT:
                tiles_nodes[b].append(int(nd))
                tile_e[b] += d
                tile_n[b] += 1
                break
        else:
            tiles_nodes.append([int(nd)])
            tile_e.append(d)
            tile_n.append(1)
    assert len(tiles_nodes) <= T_ACT * NCORES, (
        f"need {len(tiles_nodes)} tiles > {T_ACT * NCORES}"
    )

    # Tiles are striped across cores (tile g -> core g%8, local slot g//8) so
    # every core gets an equal share of dense and padding tiles: the trailing
    # all-padding tiles otherwise all land on the last core, whose gather
    # then serializes and stalls every AllGather by ~85us.
    perm = np.empty(N_NODES, np.int64)
    for t, nodes in enumerate(tiles_nodes):
        col = (t % NCORES) * NC_COLS + (t // NCORES) * SLOTS
        for j, nd in enumerate(nodes):
            perm[nd] = col + j

    q = np.zeros((NTILES, EPT, NCH, SLOTS), np.float32)
    srcslot = np.zeros((NTILES, EPT), np.int16)
    e_arange = np.arange(EPT, dtype=np.int64)
    for t in range(NTILES):
        e = 0
        for j, nd in enumerate(tiles_nodes[t] if t < len(tiles_nodes) else ()):
            s0, s1 = int(starts[nd]), int(starts[nd + 1])
            ne = s1 - s0
            if ne:
                q[t, e:e + ne, 0:4, j] = sea[s0:s1]
                q[t, e:e + ne, 4, j] = 1.0
                srcslot[t, e:e + ne] = perm[ssrc[s0:s1]].astype(np.int16)
                e += ne
        # padding rows: q is zero so the gathered values are irrelevant, but
        # give each row a distinct consecutive address -- identical indices
        # (all-zero) serialize the gather on a single HBM row
        if e < EPT:
            srcslot[t, e:] = ((t * EPT + e_arange[e:]) % NPAD).astype(np.int16)

    # channel order evens-then-odds so matmul2 can stack channel pairs
    q = q[:, :, [0, 2, 4, 1, 3], :]

    qs, idxs = [], []
    i_arange = np.arange(T * EPT)
    for k in range(NCORES):
        qt = q[k::NCORES]
        qs.append(
            np.ascontiguousarray(qt.transpose(1, 0, 2, 3)).reshape(128, T * QCT)
        )
        flat = srcslot[k::NCORES].reshape(-1)
        ia = np.zeros((128, T * 8), np.int16)
        # the index list is read per 16-partition group by each of the 8
        # GPSIMD cores on HW -> replicate it into every group
        for g in range(8):
            ia[g * 16 + i_arange % 16, i_arange // 16] = flat
        idxs.append(ia)
    return qs, idxs, perm


def _prep_inputs(inputs):
    x = np.asarray(inputs["x"], np.float32)
    qs, idxs, perm = _pack(inputs["edge_index"], inputs["edge_attr"])

    x_pad = np.zeros((NPAD, IN_F), np.float32)
    x_pad[perm] = x
    xts = [
        np.ascontiguousarray(x_pad[k * NC_COLS:(k + 1) * NC_COLS].T)
        for k in range(NCORES)
    ]

    nw = np.asarray(inputs["nn_w"], np.float32)
    wc = [nw[c].reshape(H, H) for c in range(4)]
    wc.append(np.asarray(inputs["nn_b"], np.float32).reshape(H, H))
    # matmul2 stationaries: pair p contracts evens on rows 0:64, odds on 64:128
    ws2 = np.zeros((128, 3 * H), np.float32)
    for p, (ce, co) in enumerate(((0, 1), (2, 3), (4, None))):
        ws2[0:H, p * H:(p + 1) * H] = wc[ce]
        if co is not None:
            ws2[H:128, p * H:(p + 1) * H] = wc[co]

    lin0_w = np.ascontiguousarray(np.asarray(inputs["lin0_w"], np.float32))
    root_w = np.asarray(inputs["root_w"], np.float32)
    wih_t = np.asarray(inputs["gru_w_ih"], np.float32).T  # [H, 3H]
    whh_t = np.asarray(inputs["gru_w_hh"], np.float32).T
    b_ih = np.asarray(inputs["gru_b_ih"], np.float32)
    b_hh = np.asarray(inputs["gru_b_hh"], np.float32)

    # packed 128-row stationaries vs mh = [h; m]:
    #   cols 0:128   rz gates  [whh_r|whh_z ; wih_r|wih_z]
    #   cols 128:192 n1 = wih_n @ m   [0 ; wih_n]
    #   cols 192:256 n2 = whh_n @ h   [whh_n ; 0]
    #   cols 256:320 root          [root_w ; 0]
    wpack = np.zeros((128, 5 * H), np.float32)
    wpack[0:H, 0:128] = whh_t[:, 0:128]
    wpack[H:128, 0:128] = wih_t[:, 0:128]
    wpack[H:128, 128:192] = wih_t[:, 128:192]
    wpack[0:H, 192:256] = whh_t[:, 128:192]
    wpack[0:H, 256:320] = root_w

    bias_pack = np.zeros((128, 6), np.float32)
    bias_pack[0:H, 0] = np.asarray(inputs["lin0_b"], np.float32)
    bias_pack[0:H, 1] = np.asarray(inputs["conv_b"], np.float32)
    bias_pack[0:H, 2] = (b_ih + b_hh)[0:64]
    bias_pack[0:H, 3] = b_ih[128:192]
    bias_pack[0:H, 4] = b_hh[128:192]
    bias_pack[0:H, 5] = (b_ih + b_hh)[64:128]
    ident = np.eye(128, dtype=np.float32)

    in_maps = []
    for k in range(NCORES):
        in_maps.append(
            {
                "q_in": qs[k],
                "idx_in": idxs[k],
                "xt_in": xts[k],
                "ws2_in": ws2,
                "lin0_in": lin0_w,
                "wpack_in": wpack,
                "bias_in": bias_pack,
                "ident_in": ident,
            }
        )
    return in_maps, perm


def _assemble(results, perm):
    # out_sl is column-major [H, NC_COLS]; transpose on the host
    full = np.concatenate(
        [results[k]["out_sl"].T for k in range(NCORES)], axis=0
    )
    return np.ascontiguousarray(full[perm]).astype(np.float32)


def kernel(**inputs) -> np.ndarray:
    in_maps, perm = _prep_inputs(inputs)
    nc = _get_nc()
    if os.environ.get("BASS_KERNEL_SIM"):
        results = _run_sim(nc, in_maps)
    else:
        from concourse import bass_utils

        res = bass_utils.run_bass_kernel_spmd(
            nc, in_maps, core_ids=list(range(NCORES))
        )
        results = res.results
    return _assemble(results, perm)


def _run_sim(nc, in_maps):
    from concourse.bass_interp import MultiCoreSim

    sim = MultiCoreSim(nc, num_cores=NCORES, trace=False)
    for k, core in sim.cores.items():
        for name, arr in in_maps[k].items():
            core.tensor(name)[:] = arr
    sim.simulate(check_with_hw=False)
    out = []
    for k in range(NCORES):
        out.append({"out_sl": np.array(sim.cores[k].tensor("out_sl"))})
    return out


if __name__ == "__main__":
    rng = np.random.default_rng(0)
    demo = {
        "x": rng.standard_normal((N_NODES, IN_F), dtype=np.float32),
        "edge_index": rng.integers(0, N_NODES, (2, N_EDGES)).astype(np.int32),
        "edge_attr": rng.random((N_EDGES, 4), dtype=np.float32),
        "lin0_w": rng.standard_normal((IN_F, H), dtype=np.float32) * 0.1,
        "lin0_b": np.zeros(H, np.float32),
        "nn_w": rng.standard_normal((4, H * H), dtype=np.float32) * 0.05,
        "nn_b": np.zeros(H * H, np.float32),
        "root_w": rng.standard_normal((H, H), dtype=np.float32) * 0.1,
        "conv_b": np.zeros(H, np.float32),
        "gru_w_ih": rng.standard_normal((3 * H, H), dtype=np.float32) * 0.1,
        "gru_w_hh": rng.standard_normal((3 * H, H), dtype=np.float32) * 0.1,
        "gru_b_ih": np.zeros(3 * H, np.float32),
        "gru_b_hh": np.zeros(3 * H, np.float32),
    }
    out = kernel(**demo)
    print("kernel output", out.shape, out.dtype, float(np.abs(out).mean()))


# revision 48
# speedup vs baseline: 1.0929x; 1.0928x over previous
"""Bass/Trainium2 kernel for nn_Net_19602230739296 (NNConv + GRU message passing GNN).

Algorithm (mathematically equivalent to the reference, fp32 everywhere):
  theta[e] = (edge_attr[e] @ nn_w + nn_b).reshape(H, H) is never materialized.
  msg[e]   = sum_c ea'[e,c] * (out[src_e] @ W_c)   with ea' = [edge_attr, 1],
             W_c = nn_w[c].reshape(H,H) for c<4, W_4 = nn_b.reshape(H,H).
  agg^T    = sum_c W_c^T @ (G^T @ Q_c)  per 128-edge tile, where G = out[src]
             (gathered rows) and Q_c[e, slot] = ea'[e,c] * [dst_e == slot-node]
             is a host-precomputed weighted one-hot "scatter" matrix.

Sharding: edges are sorted by destination and packed into tiles of <=128
edges covering <=32 whole destination nodes.  Nodes are renumbered to
(core, local_tile*32 + slot) with tiles STRIPED across cores (tile g ->
core g%8) so each core gets an equal share of dense and padding tiles.
Each core's edges land only in its own node range, so no cross-core
reduction is needed; the evolving node features are replicated via
AllGather each iteration.

fp32 matmul on TRN2 is two passes (LOW/HIGH), so the kernel packs pairs of
64-row contractions into single 128-row contractions wherever possible:
  - node state lives as mh = [h (rows 0:64); m (rows 64:128)], so the GRU's
    r and z gates are ONE matmul with stationary [whh_rz; wih_rz], and
    n1/n2/root use zero-padded 128-row stationaries.
  - matmul2's five channel reductions become three: S columns are stored
    even-channels (c0,c2,c4) on rows 0:64 and odd (c1,c3) on rows 64:128,
    with stationaries [W0;W1], [W2;W3], [W4;0].
"""
import os
import sys

import numpy as np


def _ensure_path():
    for p in ("/opt/trn_rl_repo", os.path.expanduser("~/.axon_site/_ro/trn_rl_repo")):
        if os.path.isdir(p) and p not in sys.path:
            sys.path.insert(0, p)
    try:
        import concourse  # noqa: F401
    except ImportError as e:  # pragma: no cover
        raise ImportError(f"concourse (bass) not importable: {e}")


_ensure_path()

N_NODES, N_EDGES, IN_F, H = 10000, 50000, 32, 64
NCORES = 8
SLOTS = 32            # destination-node slots per tile
EPT = 128             # edge slots per tile
NCH = 5               # edge_attr channels (4) + constant channel for nn_b
QCT = NCH * SLOTS     # q columns per tile (evens c0,c2,c4 then odds c1,c3)
SCT = 3 * SLOTS       # s2 columns per tile (3 channel-pairs)
T = 56                # tiles per core (fixed so the compiled NEFF is shape-stable)
T_ACT = 53            # tiles per core that can hold real edges (FFD pack: 420
                      # tiles for this graph -> ceil(420/8)=53); tiles beyond
                      # this are pure padding and the edge phase skips them
NTILES = NCORES * T   # 448
NC_COLS = T * SLOTS   # padded nodes per core (1792)
NPAD = NCORES * NC_COLS
CHUNK = 512
# Gather must be chunked: with single_packet=True the whole m2s stream of one
# dma_gather becomes one packet per SDMA engine, and the HW packet ceiling is
# 64 descriptors.  7 tiles -> 896 idxs -> 57 descs/engine.  Rotate chunks
# across the 4 SWDGE queues so descriptor generation runs on 4 Q7 cpu pairs.
GATHER_BOUNDS = (1, 2, 4, 5, 5, 5, 5, 5, 5, 6, 5, 5)  # tiles/chunk, sum=T_ACT,
# small leading chunks so matmul1 starts right after the AllGather; len
# divisible by the 4 SWDGE queues (tc locks DMASW sems per queue)
N_SWDGE_QUEUES = 4
# writeback splits here: the first half's transposes/DMA overlap the GRU tail
HALF_A = 2 * CHUNK


def _chunks():
    out = []
    c0 = 0
    while c0 < NC_COLS:
        w = min(CHUNK, NC_COLS - c0)
        out.append((c0, w))
        c0 += w
    return out


# ----------------------------------------------------------------------------
# device program
# ----------------------------------------------------------------------------
_NC_CACHE = {}


def _get_nc():
    if "nc" in _NC_CACHE:
        return _NC_CACHE["nc"]
    import concourse.bacc as bacc
    import concourse.mybir as mybir
    import concourse.tile as tile

    dt = mybir.dt
    f32, i16 = dt.float32, dt.int16
    AF = mybir.ActivationFunctionType
    ALU = mybir.AluOpType

    nc = bacc.Bacc(
        "TRN2",
        target_bir_lowering=False,
        debug=False,
        enable_asserts=False,
        num_devices=NCORES,
        num_swdge_queues=N_SWDGE_QUEUES,
    )

    q_in = nc.dram_tensor("q_in", [128, T * QCT], f32, kind="ExternalInput").ap()
    idx_in = nc.dram_tensor("idx_in", [128, T * 8], i16, kind="ExternalInput").ap()
    xt_in = nc.dram_tensor("xt_in", [IN_F, NC_COLS], f32, kind="ExternalInput").ap()
    ws2_in = nc.dram_tensor("ws2_in", [128, 3 * H], f32, kind="ExternalInput").ap()
    lin0_in = nc.dram_tensor("lin0_in", [IN_F, H], f32, kind="ExternalInput").ap()
    wpack_in = nc.dram_tensor("wpack_in", [128, 5 * H], f32, kind="ExternalInput").ap()
    bias_in = nc.dram_tensor("bias_in", [128, 6], f32, kind="ExternalInput").ap()
    ident_in = nc.dram_tensor("ident_in", [128, 128], f32, kind="ExternalInput").ap()
    out_ext = nc.dram_tensor("out_sl", [H, NC_COLS], f32, kind="ExternalOutput").ap()

    chunks = _chunks()

    with tile.TileContext(nc) as tc:
        with tc.tile_pool(name="const", bufs=1) as const, \
             tc.tile_pool(name="work", bufs=1) as work, \
             tc.tile_pool(name="small", bufs=2) as small, \
             tc.tile_pool(name="ps", bufs=4, space="PSUM") as ps, \
             tc.tile_pool(name="aggp", bufs=1, space="PSUM") as aggp, \
             tc.tile_pool(name="dram", bufs=1, space="DRAM") as dram:

            q_sb = const.tile([128, T * QCT], f32, name="q_sb")
            idx_sb = const.tile([128, T * 8], i16, name="idx_sb")
            xt_sb = const.tile([IN_F, NC_COLS], f32, name="xt_sb")
            ws2_sb = const.tile([128, 3 * H], f32, name="ws2_sb")
            lin0_sb = const.tile([IN_F, H], f32, name="lin0_sb")
            wpack_sb = const.tile([128, 5 * H], f32, name="wpack_sb")
            bias_sb = const.tile([128, 6], f32, name="bias_sb")
            ident_sb = const.tile([128, 128], f32, name="ident_sb")

            # iteration-0 dependencies first so the first AllGather can
            # trigger while the big q/idx streams are still loading
            for sb_t, in_t in (
                (xt_sb, xt_in), (lin0_sb, lin0_in), (bias_sb, bias_in),
                (ident_sb, ident_in), (ws2_sb, ws2_in), (wpack_sb, wpack_in),
                (idx_sb, idx_in), (q_sb, q_in),
            ):
                nc.sync.dma_start(sb_t[:], in_t[:])

            l0b = bias_sb[0:H, 0:1]
            convb = bias_sb[0:H, 1:2]
            br = bias_sb[0:H, 2:3]
            bnih = bias_sb[0:H, 3:4]
            bnhh = bias_sb[0:H, 4:5]
            bz = bias_sb[0:H, 5:6]

            mh_a = work.tile([128, NC_COLS], f32, name="mh_a")
            mh_b = work.tile([128, NC_COLS], f32, name="mh_b")
            row_sb = work.tile([128, (NC_COLS // 128) * H], f32, name="row_sb")
            g_sb = work.tile([128, T * H], f32, name="g_sb")
            s2_sb = work.tile([128, T * SCT], f32, name="s2_sb")

            # kill NaN risk from uninitialized SBUF that 0-stationaries touch
            # (s2 fully: the odd-channel rows of pair 2 and all inactive-tile
            # columns are never written by the per-iteration copies)
            nc.vector.memset(mh_a[64:128, :], 0.0)
            nc.vector.memset(mh_b[64:128, :], 0.0)
            nc.vector.memset(s2_sb[:], 0.0)

            agins = [dram.tile([NC_COLS, H], f32, name=f"agin{i}") for i in range(3)]
            agouts = [
                dram.tile([NPAD, H], f32, addr_space="Shared", name=f"agout{i}")
                for i in range(3)
            ]


            def writeback(mh, i, c_lo, c_hi, ag):
                # transpose h = mh[0:64] -> row-major rows [c_lo, c_hi); the
                # transposes + DMA of the first half overlap the GRU tail,
                # then one AllGather covers the full slice.  The final
                # iteration (i==3) skips the transpose entirely: out_ext is
                # column-major and the host transposes.
                if i == 3:
                    nc.sync.dma_start(out_ext[:, c_lo:c_hi], mh[0:64, c_lo:c_hi])
                    return
                for nt in range(c_lo // 128, c_hi // 128):
                    tp = ps.tile([128, H], f32, tag="w", name=f"tp{i}_{nt}")
                    nc.tensor.transpose(
                        tp[:], mh[0:64, nt * 128:(nt + 1) * 128], ident_sb[:H, :H]
                    )
                    if nt % 2 == 0:
                        nc.scalar.copy(row_sb[:, nt * H:(nt + 1) * H], tp[:])
                    else:
                        nc.vector.tensor_copy(row_sb[:, nt * H:(nt + 1) * H], tp[:])
                nc.sync.dma_start(
                    agins[i][c_lo:c_hi].rearrange("(t p) o -> p t o", p=128),
                    row_sb[:, (c_lo // 128) * H:(c_hi // 128) * H].rearrange(
                        "p (t o) -> p t o", o=H
                    ),
                )
                if ag:
                    nc.gpsimd.collective_compute(
                        "AllGather",
                        mybir.AluOpType.bypass,
                        replica_groups=[list(range(NCORES))],
                        ins=[agins[i][:]],
                        outs=[agouts[i][:]],
                    )

            # ---- iteration 0: h0 = relu(x @ lin0_w + lin0_b) ----
            for k, (c0, w) in enumerate(chunks):
                p0 = ps.tile([64, CHUNK], f32, tag="w", name=f"p0_{k}")
                nc.tensor.matmul(
                    p0[:, :w], lin0_sb[:], xt_sb[:, c0:c0 + w], start=True, stop=True
                )
                nc.scalar.activation(
                    mh_a[0:64, c0:c0 + w], p0[:, :w], AF.Relu, bias=l0b
                )
            writeback(mh_a, 0, 0, NC_COLS, ag=True)

            def edge_phase(it, mh):
                src_dram = agouts[it - 1]
                t0 = 0
                for gc, gsz in enumerate(GATHER_BOUNDS):
                    nc.gpsimd.dma_gather(
                        g_sb[:, t0 * H:(t0 + gsz) * H].rearrange(
                            "p (t o) -> p t o", o=H
                        ),
                        src_dram[:],
                        idx_sb[:, t0 * 8:(t0 + gsz) * 8],
                        gsz * EPT,
                        gsz * EPT,
                        H,
                        queue_num=gc % N_SWDGE_QUEUES,
                    )
                    t0 += gsz
                # matmul1: S_t = G_t^T @ Q_t, 3 tiles per PSUM bank.
                # S columns per tile: evens (c0,c2,c4 | 96) then odds (c1,c3 | 64);
                # the copy to s2_sb stacks odds on partitions 64:128.
                t = 0
                while t < T_ACT:
                    ntl = min(3, T_ACT - t)
                    s_ps = ps.tile([64, CHUNK], f32, tag="w", name=f"sps{it}_{t}")
                    for j in range(ntl):
                        nc.tensor.matmul(
                            s_ps[:, j * QCT:(j + 1) * QCT],
                            g_sb[:, (t + j) * H:(t + j + 1) * H],
                            q_sb[:, (t + j) * QCT:(t + j + 1) * QCT],
                            start=True,
                            stop=True,
                        )
                    src = s_ps[:, :ntl * QCT].rearrange("p (j x) -> p j x", x=QCT)
                    dste = s2_sb[0:64, t * SCT:(t + ntl) * SCT].rearrange(
                        "p (j x) -> p j x", x=SCT
                    )
                    dsto = s2_sb[64:128, t * SCT:(t + ntl) * SCT].rearrange(
                        "p (j x) -> p j x", x=SCT
                    )
                    if (t // 3) % 2 == 0:
                        nc.vector.tensor_copy(dste[:], src[:, :, 0:96])
                        nc.scalar.copy(dsto[:, :, 0:64], src[:, :, 96:160])
                    else:
                        nc.scalar.copy(dste[:], src[:, :, 0:96])
                        nc.vector.tensor_copy(dsto[:, :, 0:64], src[:, :, 96:160])
                    t += ntl
                # matmul2: agg^T += sum over 3 channel-pairs (16 tiles/matmul)
                agg = aggp.tile([64, 4 * CHUNK], f32, tag="agg", name=f"agg{it}")
                s2_re = s2_sb[:].rearrange("p (t q s) -> p t q s", q=3, s=SLOTS)
                for p in range(3):
                    g0 = 0
                    while g0 < T_ACT:
                        gn = min(16, T_ACT - g0)
                        nc.tensor.matmul(
                            agg[:, g0 * SLOTS:(g0 + gn) * SLOTS],
                            ws2_sb[:, p * H:(p + 1) * H],
                            s2_re[:, g0:g0 + gn, p, :],
                            start=(p == 0),
                            stop=False,
                        )
                        g0 += gn
                # += root_w^T h  (stationary rows 64:128 are zero -> m ignored).
                # Columns past the active-tile region got no matmul2 writes, so
                # the root matmul starts the PSUM accumulation there.
                act_end = T_ACT * SLOTS
                for c0, w in chunks:
                    if c0 + w <= act_end:
                        spans = ((c0, w, False),)
                    elif c0 >= act_end:
                        spans = ((c0, w, True),)
                    else:
                        spans = (
                            (c0, act_end - c0, False),
                            (act_end, c0 + w - act_end, True),
                        )
                    for s0, sw, st in spans:
                        nc.tensor.matmul(
                            agg[:, s0:s0 + sw], wpack_sb[:, 256:320],
                            mh[:, s0:s0 + sw], start=st, stop=True,
                        )
                return agg

            def dense_gru(agg, mh, mh_next, it, ks):
                for k in ks:
                    c0, w = chunks[k]
                    # m = relu(agg + conv_b) into the m-half of mh
                    nc.scalar.activation(
                        mh[64:128, c0:c0 + w], agg[:, c0:c0 + w], AF.Relu, bias=convb
                    )
                    # r and z gates in one 128-row matmul:
                    #   rz = [whh_rz; wih_rz]^T @ [h; m]
                    rz = ps.tile([128, CHUNK], f32, tag="w", name=f"rz{it}_{k}")
                    nc.tensor.matmul(
                        rz[:, :w], wpack_sb[:, 0:128], mh[:, c0:c0 + w],
                        start=True, stop=True,
                    )
                    r_sb = small.tile([64, CHUNK], f32, tag="rsb", name=f"rs{it}{k}")
                    nc.scalar.activation(
                        r_sb[:, :w], rz[0:64, :w], AF.Sigmoid, bias=br
                    )
                    z_sb = small.tile([64, CHUNK], f32, tag="zsb", name=f"zs{it}{k}")
                    nc.scalar.activation(
                        z_sb[:, :w], rz[64:128, :w], AF.Sigmoid, bias=bz
                    )
                    # n1 and n2 in one 128-col-stationary matmul:
                    #   n12 = [ [0;wih_n] | [whh_n;0] ]^T @ [h; m]
                    n12 = ps.tile([128, CHUNK], f32, tag="w", name=f"n12{it}_{k}")
                    nc.tensor.matmul(
                        n12[:, :w], wpack_sb[:, 128:256], mh[:, c0:c0 + w],
                        start=True, stop=True,
                    )
                    # tmp = (n2 + b_hh_n) * r
                    tmp = small.tile([64, CHUNK], f32, tag="tmp", name=f"tmp{it}{k}")
                    nc.vector.scalar_tensor_tensor(
                        tmp[:, :w], n12[64:128, :w], bnhh, r_sb[:, :w],
                        ALU.add, ALU.mult,
                    )
                    pre = small.tile([64, CHUNK], f32, tag="pre", name=f"pre{it}{k}")
                    nc.vector.tensor_add(pre[:, :w], n12[0:64, :w], tmp[:, :w])
                    nsb = small.tile([64, CHUNK], f32, tag="nsb", name=f"nsb{it}{k}")
                    nc.scalar.activation(nsb[:, :w], pre[:, :w], AF.Tanh, bias=bnih)
                    # h' = n + z * (h - n)
                    dd = small.tile([64, CHUNK], f32, tag="dd", name=f"dd{it}{k}")
                    nc.vector.tensor_sub(dd[:, :w], mh[0:64, c0:c0 + w], nsb[:, :w])
                    t4 = small.tile([64, CHUNK], f32, tag="t4", name=f"t4{it}{k}")
                    nc.vector.tensor_mul(t4[:, :w], z_sb[:, :w], dd[:, :w])
                    nc.vector.tensor_add(
                        mh_next[0:64, c0:c0 + w], nsb[:, :w], t4[:, :w]
                    )

            mh, mh_next = mh_a, mh_b
            for it in (1, 2):
                agg = edge_phase(it, mh)
                dense_gru(agg, mh, mh_next, it, (0, 1))
                writeback(mh_next, it, 0, HALF_A, ag=False)
                dense_gru(agg, mh, mh_next, it, (2, 3))
                writeback(mh_next, it, HALF_A, NC_COLS, ag=True)
                mh, mh_next = mh_next, mh
            # final iteration: per-chunk column-major DMA, no transposes
            agg = edge_phase(3, mh)
            for k, (c0, w) in enumerate(chunks):
                dense_gru(agg, mh, mh_next, 3, (k,))
                writeback(mh_next, 3, c0, c0 + w, ag=False)

    nc.compile()
    _NC_CACHE["nc"] = nc
    return nc


# ----------------------------------------------------------------------------
# host-side graph preprocessing (pure data layout, no model FLOPs)
# ----------------------------------------------------------------------------
def _pack(edge_index, edge_attr):
    src = np.asarray(edge_index[0]).astype(np.int64)
    dst = np.asarray(edge_index[1]).astype(np.int64)
    ea = np.asarray(edge_attr, np.float32)
    order = np.argsort(dst, kind="stable")
    ssrc, sea = src[order], ea[order]
    deg = np.bincount(dst, minlength=N_NODES)
    starts = np.zeros(N_NODES + 1, np.int64)
    starts[1:] = np.cumsum(deg)
    uniq = np.flatnonzero(deg)
    zs = np.flatnonzero(deg == 0)
    node_seq = np.concatenate([uniq, zs])

    # first-fit decreasing bin packing: bins capped at SLOTS nodes / EPT
    # edges; zero-degree nodes land in the node-slack of edge-full bins
    order = np.argsort(-deg[node_seq], kind="stable")
    tiles_nodes = []
    tile_e, tile_n = [], []
    for nd in node_seq[order]:
        d = int(deg[nd])
        assert d <= EPT, f"node degree {d} exceeds edge tile capacity"
        for b in range(len(tiles_nodes)):
            if tile_n[b] < SLOTS and tile_e[b] + d <= EPT:
                tiles_nodes[b].append(int(nd))
                tile_e[b] += d
                tile_n[b] += 1
                break
        else:
            tiles_nodes.append([int(nd)])
            tile_e.append(d)
            tile_n.append(1)
    assert len(tiles_nodes) <= T_ACT * NCORES, (
        f"need {len(tiles_nodes)} tiles > {T_ACT * NCORES}"
    )

    # Tiles are striped across cores (tile g -> core g%8, local slot g//8) so
    # every core gets an equal share of dense and padding tiles: the trailing
    # all-padding tiles otherwise all land on the last core, whose gather
    # then serializes and stalls every AllGather by ~85us.
    perm = np.empty(N_NODES, np.int64)
    for t, nodes in enumerate(tiles_nodes):
        col = (t % NCORES) * NC_COLS + (t // NCORES) * SLOTS
        for j, nd in enumerate(nodes):
            perm[nd] = col + j

    q = np.zeros((NTILES, EPT, NCH, SLOTS), np.float32)
    srcslot = np.zeros((NTILES, EPT), np.int16)
    e_arange = np.arange(EPT, dtype=np.int64)
    for t in range(NTILES):
        e = 0
        for j, nd in enumerate(tiles_nodes[t] if t < len(tiles_nodes) else ()):
            s0, s1 = int(starts[nd]), int(starts[nd + 1])
            ne = s1 - s0
            if ne:
                q[t, e:e + ne, 0:4, j] = sea[s0:s1]
                q[t, e:e + ne, 4, j] = 1.0
                srcslot[t, e:e + ne] = perm[ssrc[s0:s1]].astype(np.int16)
                e += ne
        # padding rows: q is zero so the gathered values are irrelevant, but
        # give each row a distinct consecutive address -- identical indices
        # (all-zero) serialize the gather on a single HBM row
        if e < EPT:
            srcslot[t, e:] = ((t * EPT + e_arange[e:]) % NPAD).astype(np.int16)

    # channel order evens-then-odds so matmul2 can stack channel pairs
    q = q[:, :, [0, 2, 4, 1, 3], :]

    qs, idxs = [], []
    i_arange = np.arange(T * EPT)
    for k in range(NCORES):
        qt = q[k::NCORES]
        qs.append(
            np.ascontiguousarray(qt.transpose(1, 0, 2, 3)).reshape(128, T * QCT)
        )
        flat = srcslot[k::NCORES].reshape(-1)
        ia = np.zeros((128, T * 8), np.int16)
        # the index list is read per 16-partition group by each of the 8
        # GPSIMD cores on HW -> replicate it into every group
        for g in range(8):
            ia[g * 16 + i_arange % 16, i_arange // 16] = flat
        idxs.append(ia)
    return qs, idxs, perm


def _prep_inputs(inputs):
    x = np.asarray(inputs["x"], np.float32)
    qs, idxs, perm = _pack(inputs["edge_index"], inputs["edge_attr"])

    x_pad = np.zeros((NPAD, IN_F), np.float32)
    x_pad[perm] = x
    xts = [
        np.ascontiguousarray(x_pad[k * NC_COLS:(k + 1) * NC_COLS].T)
        for k in range(NCORES)
    ]

    nw = np.asarray(inputs["nn_w"], np.float32)
    wc = [nw[c].reshape(H, H) for c in range(4)]
    wc.append(np.asarray(inputs["nn_b"], np.float32).reshape(H, H))
    # matmul2 stationaries: pair p contracts evens on rows 0:64, odds on 64:128
    ws2 = np.zeros((128, 3 * H), np.float32)
    for p, (ce, co) in enumerate(((0, 1), (2, 3), (4, None))):
        ws2[0:H, p * H:(p + 1) * H] = wc[ce]
        if co is not None:
            ws2[H:128, p * H:(p + 1) * H] = wc[co]

    lin0_w = np.ascontiguousarray(np.asarray(inputs["lin0_w"], np.float32))
    root_w = np.asarray(inputs["root_w"], np.float32)
    wih_t = np.asarray(inputs["gru_w_ih"], np.float32).T  # [H, 3H]
    whh_t = np.asarray(inputs["gru_w_hh"], np.float32).T
    b_ih = np.asarray(inputs["gru_b_ih"], np.float32)
    b_hh = np.asarray(inputs["gru_b_hh"], np.float32)

    # packed 128-row stationaries vs mh = [h; m]:
    #   cols 0:128   rz gates  [whh_r|whh_z ; wih_r|wih_z]
    #   cols 128:192 n1 = wih_n @ m   [0 ; wih_n]
    #   cols 192:256 n2 = whh_n @ h   [whh_n ; 0]
    #   cols 256:320 root          [root_w ; 0]
    wpack = np.zeros((128, 5 * H), np.float32)
    wpack[0:H, 0:128] = whh_t[:, 0:128]
    wpack[H:128, 0:128] = wih_t[:, 0:128]
    wpack[H:128, 128:192] = wih_t[:, 128:192]
    wpack[0:H, 192:256] = whh_t[:, 128:192]
    wpack[0:H, 256:320] = root_w

    bias_pack = np.zeros((128, 6), np.float32)
    bias_pack[0:H, 0] = np.asarray(inputs["lin0_b"], np.float32)
    bias_pack[0:H, 1] = np.asarray(inputs["conv_b"], np.float32)
    bias_pack[0:H, 2] = (b_ih + b_hh)[0:64]
    bias_pack[0:H, 3] = b_ih[128:192]
    bias_pack[0:H, 4] = b_hh[128:192]
    bias_pack[0:H, 5] = (b_ih + b_hh)[64:128]
    ident = np.eye(128, dtype=np.float32)

    in_maps = []
    for k in range(NCORES):
        in_maps.append(
            {
                "q_in": qs[k],
                "idx_in": idxs[k],
                "xt_in": xts[k],
                "ws2_in": ws2,
                "lin0_in": lin0_w,
                "wpack_in": wpack,
                "bias_in": bias_pack,
                "ident_in": ident,
            }
        )
    return in_maps, perm


def _assemble(results, perm):
    # out_sl is column-major [H, NC_COLS]; transpose on the host
    full = np.concatenate(
        [results[k]["out_sl"].T for k in range(NCORES)], axis=0
    )
    return np.ascontiguousarray(full[perm]).astype(np.float32)


def kernel(**inputs) -> np.ndarray:
    in_maps, perm = _prep_inputs(inputs)
    nc = _get_nc()
    if os.environ.get("BASS_KERNEL_SIM"):
        results = _run_sim(nc, in_maps)
    else:
        from concourse import bass_utils

        res = bass_utils.run_bass_kernel_spmd(
            nc, in_maps, core_ids=list(range(NCORES))
        )
        results = res.results
    return _assemble(results, perm)


def _run_sim(nc, in_maps):
    from concourse.bass_interp import MultiCoreSim

    sim = MultiCoreSim(nc, num_cores=NCORES, trace=False)
    for k, core in sim.cores.items():
        for name, arr in in_maps[k].items():
            core.tensor(name)[:] = arr
    sim.simulate(check_with_hw=False)
    out = []
    for k in range(NCORES):
        out.append({"out_sl": np.array(sim.cores[k].tensor("out_sl"))})
    return out


if __name__ == "__main__":
    rng = np.random.default_rng(0)
    demo = {
        "x": rng.standard_normal((N_NODES, IN_F), dtype=np.float32),
        "edge_index": rng.integers(0, N_NODES, (2, N_EDGES)).astype(np.int32),
        "edge_attr": rng.random((N_EDGES, 4), dtype=np.float32),
        "lin0_w": rng.standard_normal((IN_F, H), dtype=np.float32) * 0.1,
        "lin0_b": np.zeros(H, np.float32),
        "nn_w": rng.standard_normal((4, H * H), dtype=np.float32) * 0.05,
        "nn_b": np.zeros(H * H, np.float32),
        "root_w": rng.standard_normal((H, H), dtype=np.float32) * 0.1,
        "conv_b": np.zeros(H, np.float32),
        "gru_w_ih": rng.standard_normal((3 * H, H), dtype=np.float32) * 0.1,
        "gru_w_hh": rng.standard_normal((3 * H, H), dtype=np.float32) * 0.1,
        "gru_b_ih": np.zeros(3 * H, np.float32),
        "gru_b_hh": np.zeros(3 * H, np.float32),
    }
    out = kernel(**demo)
    print("kernel output", out.shape, out.dtype, float(np.abs(out).mean()))
